# revision 5
# baseline (speedup 1.0000x reference)
"""Trainium2 Bass kernel for the HardResetSSMBlock problem.

y = silu(x @ W1 + b1) @ W2 + b2, masked per frame, with a periodic decay
scale on frames where (t+1) % 10 == 0.

The op is memory-bound: 134 MB in + 134 MB out at f32. Two structural
cuts get the device traffic down ~4x vs the dense-f32 version:

1. The mask zeroes ~half the frames, and the host knows the mask. Only
   unmasked tokens are packed (host gather) and shipped; the result is
   scattered back into a zeros array on host. The per-token decay scale
   is likewise applied during the host-side scatter, so the device needs
   no mask/scale input at all.
2. bf16 on the wire in both directions (and for the weights). Verified
   rel-err of the bf16 pipeline vs the f32 reference is ~4e-3 against a
   tolerance of 2e-2.

Device program per 2048-token tile (8 cores, data-parallel over packed
tokens; tile count adapts to the mask popcount, cached per NEFF):
  DMA in (x^T, feature-major bf16) -> 4x matmul [128,512] with W1
  stationary -> Silu(+b1) on ACT in 1024-col halves (f32 PSUM -> bf16
  SBUF) -> 4x matmul with W2 stationary (y stays feature-major, no
  per-chunk stationary reloads) -> DVE copy PSUM -> bf16 SBUF -> DMA out.
PSUM: 2 bufs x [128,1024] for MM1 + 2 x [128,1024] for MM2 = all 8 banks.
"""

import numpy as np

B, S, D = 16, 16384, 128
RESET_PERIOD = 10
DECAY_FACTOR = 0.1
N_CORES = 8
TILE_TOK = 2048
BLK = 512   # one matmul output = one PSUM bank
HALF = 1024  # ACT / DVE instruction granularity (2 banks)

ACT_FUNC = "Silu"

_CACHE = {}


def _build_nc(n_tiles):
    import concourse.bacc as bacc
    import concourse.tile as tile
    from concourse import mybir

    f32 = mybir.dt.float32
    bf16 = mybir.dt.bfloat16

    nc = bacc.Bacc()
    xt_d = nc.dram_tensor(
        "x_t", [n_tiles, 128, TILE_TOK], bf16, kind="ExternalInput"
    )
    w1_d = nc.dram_tensor("w1", [D, D], bf16, kind="ExternalInput")
    w2_d = nc.dram_tensor("w2", [D, D], bf16, kind="ExternalInput")
    b1_d = nc.dram_tensor("b1", [D, 1], f32, kind="ExternalInput")
    # feature-major output tiles: y_t[t, e, j] = y[t*TILE_TOK + j, e]
    y_d = nc.dram_tensor(
        "y_t", [n_tiles, 128, TILE_TOK], bf16, kind="ExternalOutput"
    )

    act = getattr(mybir.ActivationFunctionType, ACT_FUNC)

    with tile.TileContext(nc) as tc:
        with (
            tc.tile_pool(name="const", bufs=1) as constp,
            tc.tile_pool(name="xt", bufs=6) as xtp,
            tc.tile_pool(name="ht", bufs=3) as htp,
            tc.tile_pool(name="yout", bufs=4) as youtp,
            tc.tile_pool(name="ps_h", bufs=2, space="PSUM") as ps_hp,
            tc.tile_pool(name="ps_y", bufs=2, space="PSUM") as ps_yp,
        ):
            w1_s = constp.tile([128, 128], bf16)
            nc.gpsimd.dma_start(w1_s[:], w1_d[:])
            w2_s = constp.tile([128, 128], bf16)
            nc.gpsimd.dma_start(w2_s[:], w2_d[:])
            b1_s = constp.tile([128, 1], f32)
            nc.gpsimd.dma_start(b1_s[:], b1_d[:])

            for t in range(n_tiles):
                s_xt = xtp.tile([128, TILE_TOK], bf16)
                # all ins on the SP HWDGE queue, all outs on the ACT one:
                # a queue is FIFO, so mixing directions head-of-line-blocks
                # prefetches behind compute-gated stores
                nc.sync.dma_start(s_xt[:], xt_d[t])

                s_ht = htp.tile([128, TILE_TOK], bf16)
                s_y = youtp.tile([128, TILE_TOK], bf16)
                for hf in range(TILE_TOK // HALF):
                    p_h = ps_hp.tile([128, HALF], f32)
                    for bk in range(HALF // BLK):
                        lo = hf * HALF + bk * BLK
                        nc.tensor.matmul(
                            p_h[:, bk * BLK:(bk + 1) * BLK],
                            w1_s[:], s_xt[:, lo:lo + BLK],
                            start=True, stop=True,
                        )
                    nc.scalar.activation(
                        s_ht[:, hf * HALF:(hf + 1) * HALF], p_h[:],
                        act, bias=b1_s[:], scale=1.0,
                    )
                for hf in range(TILE_TOK // HALF):
                    p_y = ps_yp.tile([128, HALF], f32)
                    for bk in range(HALF // BLK):
                        lo = hf * HALF + bk * BLK
                        nc.tensor.matmul(
                            p_y[:, bk * BLK:(bk + 1) * BLK],
                            w2_s[:], s_ht[:, lo:lo + BLK],
                            start=True, stop=True,
                        )
                    nc.vector.tensor_copy(
                        s_y[:, hf * HALF:(hf + 1) * HALF], p_y[:]
                    )

                nc.scalar.dma_start(y_d[t], s_y[:])

    nc.finalize()
    return nc


def _get_nc(n_tiles):
    key = ("nc", n_tiles)
    if key not in _CACHE:
        _CACHE[key] = _build_nc(n_tiles)
    return _CACHE[key]


def kernel(x, mask, W1, b1, W2, b2, _trace=False):
    from concourse import mybir
    from concourse.bass_utils import run_bass_kernel_spmd

    bf16 = mybir.dt.np(mybir.dt.bfloat16)

    x = np.asarray(x, dtype=np.float32)
    mask = np.asarray(mask).astype(bool)
    W1 = np.asarray(W1, dtype=np.float32)
    W2 = np.asarray(W2, dtype=np.float32)
    b1 = np.asarray(b1, dtype=np.float32)
    b2 = np.asarray(b2, dtype=np.float32)

    Bx, Sx, Dx = x.shape
    x_flat = np.ascontiguousarray(x).reshape(Bx * Sx, Dx)
    idx = np.flatnonzero(mask.ravel())
    count = idx.size

    t = np.arange(Sx)
    decay = np.where(
        (t + 1) % RESET_PERIOD == 0, DECAY_FACTOR, 1.0
    ).astype(np.float32)

    out = np.zeros((Bx * Sx, Dx), dtype=np.float32)

    if count > 0:
        n_tiles = -(-count // (N_CORES * TILE_TOK))
        cap = n_tiles * TILE_TOK
        total = cap * N_CORES

        xg = np.zeros((total, Dx), dtype=bf16)
        xg[:count] = x_flat[idx].astype(bf16)
        # per-core feature-major tiles [n_tiles, D, TILE_TOK]
        x_t = np.ascontiguousarray(
            xg.reshape(N_CORES, n_tiles, TILE_TOK, Dx).transpose(0, 1, 3, 2)
        )

        w1b = np.ascontiguousarray(W1.astype(bf16))
        w2b = np.ascontiguousarray(W2.astype(bf16))
        b1c = np.ascontiguousarray(b1.reshape(Dx, 1))

        nc = _get_nc(n_tiles)
        in_maps = [
            {"x_t": x_t[c], "w1": w1b, "w2": w2b, "b1": b1c}
            for c in range(N_CORES)
        ]
        res = run_bass_kernel_spmd(
            nc, in_maps, list(range(N_CORES)), trace=_trace
        )
        if _trace:
            _CACHE["last_results"] = res

        # y_t [n_tiles, D, TILE_TOK] feature-major -> token-major rows
        y_all = np.stack([res.results[c]["y_t"] for c in range(N_CORES)])
        y_tok = np.ascontiguousarray(
            y_all.transpose(0, 1, 3, 2)
        ).reshape(total, Dx)
        dec_flat = np.broadcast_to(decay, (Bx, Sx)).reshape(-1)
        out[idx] = y_tok[:count].astype(np.float32) * dec_flat[idx][:, None]

    out = out.reshape(Bx, Sx, Dx)
    if np.any(b2):
        # device computes h @ W2 without b2; reference adds b2 before the
        # mask/decay scaling, so fold it in here on the host
        s_full = mask.astype(np.float32) * decay[None, :]
        out = out + s_full[:, :, None] * b2[None, None, :]
    return out


# revision 7
# speedup vs baseline: 1.1628x; 1.1628x over previous
"""Trainium2 Bass kernel for the HardResetSSMBlock problem.

y = silu(x @ W1 + b1) @ W2 + b2, masked per frame, with a periodic decay
scale on frames where (t+1) % 10 == 0.

The op is memory-bound: 134 MB in + 134 MB out at f32. Two structural
cuts get the device traffic down ~4x vs the dense-f32 version:

1. The mask zeroes ~half the frames, and the host knows the mask. Only
   unmasked tokens are packed (host gather) and shipped; the result is
   scattered back into a zeros array on host. The per-token decay scale
   is likewise applied during the host-side scatter, so the device needs
   no mask/scale input at all.
2. bf16 on the wire in both directions (and for the weights). Verified
   rel-err of the bf16 pipeline vs the f32 reference is ~4e-3 against a
   tolerance of 2e-2.

Device program per 2048-token tile (8 cores, data-parallel over packed
tokens; tile count adapts to the mask popcount, cached per NEFF):
  DMA in (x^T, feature-major bf16) -> 4x matmul [128,512] with W1
  stationary -> Silu(+b1) on ACT in 1024-col halves (f32 PSUM -> bf16
  SBUF) -> 4x matmul with W2 stationary (y stays feature-major, no
  per-chunk stationary reloads) -> DVE copy PSUM -> bf16 SBUF -> DMA out.
PSUM: 2 bufs x [128,1024] for MM1 + 2 x [128,1024] for MM2 = all 8 banks.
"""

import numpy as np

B, S, D = 16, 16384, 128
RESET_PERIOD = 10
DECAY_FACTOR = 0.1
N_CORES = 8
TILE_TOK = 2048
BLK = 512   # one matmul output = one PSUM bank
HALF = 1024  # ACT / DVE instruction granularity (2 banks)

ACT_FUNC = "Silu"

_CACHE = {}


def _build_nc(n_tiles):
    import concourse.bacc as bacc
    import concourse.tile as tile
    from concourse import mybir

    f32 = mybir.dt.float32
    bf16 = mybir.dt.bfloat16

    nc = bacc.Bacc()
    xt_d = nc.dram_tensor(
        "x_t", [n_tiles, 128, TILE_TOK], bf16, kind="ExternalInput"
    )
    w1_d = nc.dram_tensor("w1", [D, D], bf16, kind="ExternalInput")
    w2_d = nc.dram_tensor("w2", [D, D], bf16, kind="ExternalInput")
    b1_d = nc.dram_tensor("b1", [D, 1], f32, kind="ExternalInput")
    # feature-major output tiles: y_t[t, e, j] = y[t*TILE_TOK + j, e]
    y_d = nc.dram_tensor(
        "y_t", [n_tiles, 128, TILE_TOK], bf16, kind="ExternalOutput"
    )

    act = getattr(mybir.ActivationFunctionType, ACT_FUNC)

    with tile.TileContext(nc) as tc:
        with (
            tc.tile_pool(name="const", bufs=1) as constp,
            tc.tile_pool(name="xt", bufs=n_tiles) as xtp,
            tc.tile_pool(name="ht", bufs=3) as htp,
            tc.tile_pool(name="yout", bufs=4) as youtp,
            tc.tile_pool(name="ps_h", bufs=2, space="PSUM") as ps_hp,
            tc.tile_pool(name="ps_y", bufs=2, space="PSUM") as ps_yp,
        ):
            # consts + every in-DMA go on the sync HWDGE queue, triggered
            # upfront with no semaphore waits (bufs=n_tiles keeps the whole
            # input resident), so the read stream runs at full rate.  DMA
            # trigger instructions occupy the issuing engine ~0.7us each
            # and block its stream on their waits, so outs go on gpsimd
            # (SWDGE), which is otherwise idle — scalar runs only ACTs and
            # never stalls on a store.
            w1_s = constp.tile([128, 128], bf16)
            nc.sync.dma_start(w1_s[:], w1_d[:])
            w2_s = constp.tile([128, 128], bf16)
            nc.sync.dma_start(w2_s[:], w2_d[:])
            b1_s = constp.tile([128, 1], f32)
            nc.sync.dma_start(b1_s[:], b1_d[:])

            s_xts = []
            for t in range(n_tiles):
                s_xt = xtp.tile([128, TILE_TOK], bf16)
                nc.sync.dma_start(s_xt[:], xt_d[t])
                s_xts.append(s_xt)

            for t in range(n_tiles):
                s_xt = s_xts[t]
                s_ht = htp.tile([128, TILE_TOK], bf16)
                s_y = youtp.tile([128, TILE_TOK], bf16)
                for hf in range(TILE_TOK // HALF):
                    p_h = ps_hp.tile([128, HALF], f32)
                    for bk in range(HALF // BLK):
                        lo = hf * HALF + bk * BLK
                        nc.tensor.matmul(
                            p_h[:, bk * BLK:(bk + 1) * BLK],
                            w1_s[:], s_xt[:, lo:lo + BLK],
                            start=True, stop=True,
                        )
                    nc.scalar.activation(
                        s_ht[:, hf * HALF:(hf + 1) * HALF], p_h[:],
                        act, bias=b1_s[:], scale=1.0,
                    )
                for hf in range(TILE_TOK // HALF):
                    p_y = ps_yp.tile([128, HALF], f32)
                    for bk in range(HALF // BLK):
                        lo = hf * HALF + bk * BLK
                        nc.tensor.matmul(
                            p_y[:, bk * BLK:(bk + 1) * BLK],
                            w2_s[:], s_ht[:, lo:lo + BLK],
                            start=True, stop=True,
                        )
                    nc.vector.tensor_copy(
                        s_y[:, hf * HALF:(hf + 1) * HALF], p_y[:]
                    )

                nc.gpsimd.dma_start(y_d[t], s_y[:])

    nc.finalize()
    return nc


def _get_nc(n_tiles):
    key = ("nc", n_tiles)
    if key not in _CACHE:
        _CACHE[key] = _build_nc(n_tiles)
    return _CACHE[key]


def kernel(x, mask, W1, b1, W2, b2, _trace=False):
    from concourse import mybir
    from concourse.bass_utils import run_bass_kernel_spmd

    bf16 = mybir.dt.np(mybir.dt.bfloat16)

    x = np.asarray(x, dtype=np.float32)
    mask = np.asarray(mask).astype(bool)
    W1 = np.asarray(W1, dtype=np.float32)
    W2 = np.asarray(W2, dtype=np.float32)
    b1 = np.asarray(b1, dtype=np.float32)
    b2 = np.asarray(b2, dtype=np.float32)

    Bx, Sx, Dx = x.shape
    x_flat = np.ascontiguousarray(x).reshape(Bx * Sx, Dx)
    idx = np.flatnonzero(mask.ravel())
    count = idx.size

    t = np.arange(Sx)
    decay = np.where(
        (t + 1) % RESET_PERIOD == 0, DECAY_FACTOR, 1.0
    ).astype(np.float32)

    out = np.zeros((Bx * Sx, Dx), dtype=np.float32)

    if count > 0:
        n_tiles = -(-count // (N_CORES * TILE_TOK))
        cap = n_tiles * TILE_TOK
        total = cap * N_CORES

        xg = np.zeros((total, Dx), dtype=bf16)
        xg[:count] = x_flat[idx].astype(bf16)
        # per-core feature-major tiles [n_tiles, D, TILE_TOK]
        x_t = np.ascontiguousarray(
            xg.reshape(N_CORES, n_tiles, TILE_TOK, Dx).transpose(0, 1, 3, 2)
        )

        w1b = np.ascontiguousarray(W1.astype(bf16))
        w2b = np.ascontiguousarray(W2.astype(bf16))
        b1c = np.ascontiguousarray(b1.reshape(Dx, 1))

        nc = _get_nc(n_tiles)
        in_maps = [
            {"x_t": x_t[c], "w1": w1b, "w2": w2b, "b1": b1c}
            for c in range(N_CORES)
        ]
        res = run_bass_kernel_spmd(
            nc, in_maps, list(range(N_CORES)), trace=_trace
        )
        if _trace:
            _CACHE["last_results"] = res

        # y_t [n_tiles, D, TILE_TOK] feature-major -> token-major rows
        y_all = np.stack([res.results[c]["y_t"] for c in range(N_CORES)])
        y_tok = np.ascontiguousarray(
            y_all.transpose(0, 1, 3, 2)
        ).reshape(total, Dx)
        dec_flat = np.broadcast_to(decay, (Bx, Sx)).reshape(-1)
        out[idx] = y_tok[:count].astype(np.float32) * dec_flat[idx][:, None]

    out = out.reshape(Bx, Sx, Dx)
    if np.any(b2):
        # device computes h @ W2 without b2; reference adds b2 before the
        # mask/decay scaling, so fold it in here on the host
        s_full = mask.astype(np.float32) * decay[None, :]
        out = out + s_full[:, :, None] * b2[None, None, :]
    return out
